# revision 1
# baseline (speedup 1.0000x reference)
"""GroupedQueryAttention, tensor-parallel over heads on 8 NeuronCores (raw Bass).

Core c owns q heads {2c, 2c+1} (q cols c*128..c*128+128) and kv head c//2
(kv cols (c//2)*64..+64). Device pipeline (transposed layout: feature dim on
partitions, sequence on free axis):
  hsT = hs.T (PE transposes) -> qT/kT/vT projections (PE, f32r) ->
  RoPE (DVE, swap-halves via DMA) ->
  per head: S^T[k,q] = kT_tile.T @ qT (PE) -> P^T = exp(0.125 S^T) (ACT) ->
  outT = [V|1].T @ P^T (PE, fused denominator row) -> normalize (DVE) ->
  AllGather(attnT) -> out_sliceT = Wo_slice_tiles.T @ attnT_full (PE).
Host: out[:, c*128:(c+1)*128] = out_t_c.T ; reshape to [1, S, HID].
"""
import sys, os
sys.path.insert(0, '/opt/trn_rl_repo')
import contextlib
import numpy as np
import concourse.bass as bass
import concourse.mybir as mybir
from concourse.bass_utils import run_bass_kernel_spmd

F32 = mybir.dt.float32
F32R = mybir.dt.float32r
EXP = mybir.ActivationFunctionType.Exp

S, HID, HD = 2048, 1024, 64
NCORES = 8
NST = S // 128      # 16
NHT = HID // 128    # 8
NSC = 4             # 512-wide chunks
NPC = 8             # 256-wide hs transpose pieces


def build_kernel():
    nc = bass.Bass("TRN2", target_bir_lowering=False, num_devices=NCORES)

    hs_d = nc.dram_tensor("hs", [S, HID], F32, kind="ExternalInput")
    wq_d = nc.dram_tensor("wq", [HID, 128], F32, kind="ExternalInput")
    wk_d = nc.dram_tensor("wk", [HID, HD], F32, kind="ExternalInput")
    wv_d = nc.dram_tensor("wv", [HID, HD], F32, kind="ExternalInput")
    wo_d = nc.dram_tensor("wo", [HID, 128], F32, kind="ExternalInput")
    cs_d = nc.dram_tensor("cs", [S, 128], F32, kind="ExternalInput")
    id_d = nc.dram_tensor("ident", [128, 128], F32, kind="ExternalInput")
    out_d = nc.dram_tensor("out_t", [128, S], F32, kind="ExternalOutput")
    scr_d = nc.dram_tensor("scr", [512], F32)
    agin_d = nc.dram_tensor("agin", [2, HD, S], F32R)
    agout_d = nc.dram_tensor("agout", [NCORES, 2, HD, S], F32R, addr_space="Shared")

    def sb(name, shape, dt):
        return nc.alloc_sbuf_tensor(name, shape, dt).ap()

    hs_ch = [sb(f"hs_ch{i}", [128, 2, HID], F32) for i in range(2)]
    hsT_ch = [sb(f"hsT_ch{i}", [128, NHT, 512], F32R) for i in range(2)]
    ident = sb("ident_sb", [128, 128], F32)
    cs_st = sb("cs_st", [128, NST, 128], F32)
    cosT = sb("cosT", [HD, S], F32)
    ssinT = sb("ssinT", [HD, S], F32)
    wq_sb = sb("wq_sb", [128, NHT, 128], F32)
    wk_sb = sb("wk_sb", [128, NHT, HD], F32)
    wv_sb = sb("wv_sb", [128, NHT, HD], F32)
    wo_sb = sb("wo_sb", [128, NHT, 128], F32)
    wq_r = sb("wq_r", [128, NHT, 128], F32R)
    wk_r = sb("wk_r", [128, NHT, HD], F32R)
    wv_r = sb("wv_r", [128, NHT, HD], F32R)
    wo_r = sb("wo_r", [128, NHT, 128], F32R)
    q_ch = [[sb(f"q_ch{h}_{i}", [HD, 512], F32) for i in range(2)] for h in range(2)]
    qs_ch = [[sb(f"qs_ch{h}_{i}", [HD, 512], F32) for i in range(2)] for h in range(2)]
    k_ch = [sb(f"k_ch{i}", [HD, 512], F32) for i in range(2)]
    ks_ch = [sb(f"ks_ch{i}", [HD, 512], F32) for i in range(2)]
    tmp_a = sb("tmp_a", [HD, 512], F32)
    tmp_b = sb("tmp_b", [HD, 512], F32)
    qT_r = [sb(f"qT_r{h}", [HD, S], F32R) for h in range(2)]
    kT_r = sb("kT_r", [HD, S], F32R)
    vT = sb("vT", [HD, S], F32)
    vaug = sb("vaug", [128, NST, HD + 1], F32R)
    PT = [sb(f"PT{i}", [128, 1024], F32R) for i in range(2)]
    misc = sb("misc", [128, 512], F32)
    rb_ch = sb("rb_ch", [HD, 512], F32)
    af_ch = sb("af_ch", [128, NHT, 512], F32R)
    out_ch = [sb(f"out_ch{i}", [128, 512], F32) for i in range(2)]

    ps = nc.alloc_psum_tensor("psblob", [128, 4096], F32).ap()
    tp_p = [ps[:, 0:256], ps[:, 512:768]]
    qp_p = [ps[0:HD, 1024:1536], ps[0:HD, 1536:2048]]
    kp_p = ps[0:HD, 2048:2560]
    vp_p = ps[0:HD, 2560:3072]
    cs_p = [ps[:, 3072:3328], ps[:, 3584:3840]]
    sp_p = [ps[:, 0:1024], ps[:, 1024:2048]]
    pv_p = ps[0:HD + 1, 2048:4096]
    op_p = [ps[:, 0:512], ps[:, 512:1024]]

    es = contextlib.ExitStack()
    SEM = lambda n: es.enter_context(nc.semaphore(n))
    sL = SEM("sL")      # SP input DMAs (+16)
    sTP = SEM("sTP")    # PE hs-transpose groups (64)
    sHST = SEM("sHST")  # DVE hsT copies (64)
    sWR = SEM("sWR")    # DVE weight rounding (4)
    sCS = SEM("sCS")    # PE cos/sin transpose fills (16)
    sCSC = SEM("sCSC")  # DVE cos/sin copies (16)
    sMS = SEM("sMS")    # PL ones memset (1)
    sQP0 = SEM("sQP0"); sQP1 = SEM("sQP1"); sKP = SEM("sKP"); sVP = SEM("sVP")
    sQC = SEM("sQC"); sKC = SEM("sKC"); sVC = SEM("sVC")
    sSW = SEM("sSW")    # ACT-issued swap DMAs (+16 each, 24 total)
    sRQ0 = SEM("sRQ0"); sRQ1 = SEM("sRQ1"); sRK = SEM("sRK")
    sVT = SEM("sVT"); sVA = SEM("sVA")
    sSC = SEM("sSC"); sEX = SEM("sEX"); sPV = SEM("sPV")
    sRC = SEM("sRC"); sRBD = SEM("sRBD"); sRB = SEM("sRB"); sNMC = SEM("sNMC")
    sAGI = SEM("sAGI"); sCC = SEM("sCC"); sAF = SEM("sAF")
    sOP = SEM("sOP"); sOC = SEM("sOC"); sOD = SEM("sOD")
    sQP = [sQP0, sQP1]
    sRQ = [sRQ0, sRQ1]
    sR3 = [sRQ0, sRQ1, sRK]

    def _pv(tensor, h, kt):
        tensor.wait_ge(sEX, h * 32 + 2 * kt + 2)
        if h == 1 and kt == 0:
            tensor.wait_ge(sNMC, 4)  # head0 normalize done, pv psum free
        for qc in range(4):
            inst = tensor.matmul(
                pv_p[:, qc * 512:(qc + 1) * 512],
                vaug[:, kt, :],
                PT[qc // 2][:, (qc % 2) * 512:(qc % 2 + 1) * 512],
                start=(kt == 0), stop=(kt == NST - 1),
            )
        inst.then_inc(sPV, 1)

    with nc.Block() as block:

        # ================= SP: loads + stores =================
        @block.sync
        def _(sync):
            sync.dma_start(out=ident, in_=id_d[:]).then_inc(sL, 16)
            sync.dma_start(out=wq_sb, in_=wq_d[:].rearrange("(t p) c -> p t c", p=128)).then_inc(sL, 16)
            sync.dma_start(out=wk_sb, in_=wk_d[:].rearrange("(t p) c -> p t c", p=128)).then_inc(sL, 16)
            sync.dma_start(out=wv_sb, in_=wv_d[:].rearrange("(t p) c -> p t c", p=128)).then_inc(sL, 16)
            sync.dma_start(out=wo_sb, in_=wo_d[:].rearrange("(t p) c -> p t c", p=128)).then_inc(sL, 16)
            sync.dma_start(out=cs_st, in_=cs_d[:].rearrange("(t p) c -> p t c", p=128)).then_inc(sL, 16)
            for pc in range(NPC):
                if pc >= 2:
                    sync.wait_ge(sTP, (pc - 1) * NHT)  # piece pc-2 fully transposed
                sync.dma_start(
                    out=hs_ch[pc % 2],
                    in_=hs_d[:].rearrange("(t p) h -> p t h", p=128)[:, 2 * pc:2 * pc + 2, :],
                ).then_inc(sL, 16)
            # ---- o_proj phase ----
            sync.wait_ge(sCC, 1)
            for oc in range(NSC):
                if oc >= 1:
                    sync.wait_ge(sOP, oc)
                sync.dma_start(
                    out=af_ch,
                    in_=agout_d[:].rearrange("c h d (n q) -> (h d) c n q", q=512)[:, :, oc, :],
                ).then_inc(sAF, 16)
            for oc in range(NSC):
                sync.wait_ge(sOC, oc + 1)
                sync.dma_start(out=out_d[:, oc * 512:(oc + 1) * 512], in_=out_ch[oc % 2]).then_inc(sOD, 16)
            sync.wait_ge(sOD, 16 * NSC)

        # ================= PE =================
        @block.tensor
        def _(tensor):
            # cos/sin transposes: 16 fills of [64, 256] (2 seqtiles each)
            tensor.wait_ge(sL, 96)
            for f in range(16):
                is_sin = f >= 8
                pr = f % 8
                c0 = 64 if is_sin else 0
                if f >= 2:
                    tensor.wait_ge(sCSC, f - 1)
                for j in range(2):
                    st = 2 * pr + j
                    inst = tensor.transpose(
                        cs_p[f % 2][0:HD, j * 128:(j + 1) * 128],
                        cs_st[:, st, c0:c0 + HD],
                        ident,
                    )
                inst.then_inc(sCS, 1)
            # per 512-chunk: hs transposes then projections then v transposes
            tensor.wait_ge(sWR, 4)
            for sc in range(NSC):
                for pc in (2 * sc, 2 * sc + 1):
                    tensor.wait_ge(sL, 96 + 16 * (pc + 1))
                    for ht in range(NHT):
                        g = pc * NHT + ht
                        if g >= 2:
                            tensor.wait_ge(sHST, g - 1)
                        for st in range(2):
                            inst = tensor.transpose(
                                tp_p[g % 2][:, st * 128:(st + 1) * 128],
                                hs_ch[pc % 2][:, st, ht * 128:(ht + 1) * 128],
                                ident,
                            )
                        inst.then_inc(sTP, 1)
                tensor.wait_ge(sHST, (2 * sc + 2) * NHT)
                for h in range(2):
                    if sc >= 1:
                        tensor.wait_ge(sQC, 2 * (sc - 1) + h + 1)
                    for ht in range(NHT):
                        inst = tensor.matmul(
                            qp_p[h], wq_r[:, ht, h * HD:(h + 1) * HD],
                            hsT_ch[sc % 2][:, ht, :],
                            start=(ht == 0), stop=(ht == NHT - 1),
                        )
                    inst.then_inc(sQP[h], 1)
                if sc >= 1:
                    tensor.wait_ge(sKC, sc)
                for ht in range(NHT):
                    inst = tensor.matmul(
                        kp_p, wk_r[:, ht, :], hsT_ch[sc % 2][:, ht, :],
                        start=(ht == 0), stop=(ht == NHT - 1),
                    )
                inst.then_inc(sKP, 1)
                if sc >= 1:
                    tensor.wait_ge(sVC, sc)
                for ht in range(NHT):
                    inst = tensor.matmul(
                        vp_p, wv_r[:, ht, :], hsT_ch[sc % 2][:, ht, :],
                        start=(ht == 0), stop=(ht == NHT - 1),
                    )
                inst.then_inc(sVP, 1)
                # v transposes for this chunk (4 ktiles) -> [128, 64] psum
                tensor.wait_ge(sVC, sc + 1)
                if sc == 0:
                    tensor.wait_ge(sCSC, 16)  # cs psum banks drained
                for j in range(4):
                    kt = 4 * sc + j
                    if kt >= 2:
                        tensor.wait_ge(sVA, kt - 1)
                    inst = tensor.transpose(
                        cs_p[kt % 2][0:128, 0:HD],
                        vT[:, kt * 128:(kt + 1) * 128],
                        ident[0:HD, 0:HD],
                    )
                    inst.then_inc(sVT, 1)
            # ================= attention =================
            tensor.wait_ge(sRK, NSC)
            tensor.wait_ge(sVA, NST)
            tensor.wait_ge(sMS, 1)
            for h in range(2):
                tensor.wait_ge(sRQ[h], NSC)
                for kt in range(NST):
                    u_base = h * 32
                    for qh in range(2):
                        u = kt * 2 + qh
                        if u_base + u >= 2:
                            tensor.wait_ge(sEX, u_base + u - 1)
                        for i in range(2):
                            inst = tensor.matmul(
                                sp_p[u % 2][:, i * 512:(i + 1) * 512],
                                kT_r[:, kt * 128:(kt + 1) * 128],
                                qT_r[h][:, qh * 1024 + i * 512: qh * 1024 + (i + 1) * 512],
                                start=True, stop=True,
                            )
                        inst.then_inc(sSC, 1)
                    # PV for kt-1 (pipeline lag so ACT keeps up)
                    if kt >= 1:
                        _pv(tensor, h, kt - 1)
                _pv(tensor, h, NST - 1)
            # ================= o_proj =================
            tensor.wait_ge(sEX, 64)
            for oc in range(NSC):
                tensor.wait_ge(sAF, 16 * (oc + 1))
                if oc >= 2:
                    tensor.wait_ge(sOC, oc - 1)
                for ht in range(NHT):
                    inst = tensor.matmul(
                        op_p[oc % 2], wo_r[:, ht, :], af_ch[:, ht, :],
                        start=(ht == 0), stop=(ht == NHT - 1),
                    )
                inst.then_inc(sOP, 1)

        # ================= DVE =================
        @block.vector
        def _(vector):
            vector.wait_ge(sL, 80)
            vector.tensor_copy(wq_r, wq_sb).then_inc(sWR, 1)
            vector.tensor_copy(wk_r, wk_sb).then_inc(sWR, 1)
            vector.tensor_copy(wv_r, wv_sb).then_inc(sWR, 1)
            vector.tensor_copy(wo_r, wo_sb).then_inc(sWR, 1)
            # cos/sin copies
            for f in range(16):
                is_sin = f >= 8
                pr = f % 8
                vector.wait_ge(sCS, f + 1)
                if not is_sin:
                    vector.tensor_copy(
                        cosT[:, pr * 256:(pr + 1) * 256], cs_p[f % 2][0:HD, 0:256]
                    ).then_inc(sCSC, 1)
                else:
                    vector.tensor_scalar_mul(
                        ssinT[0:32, pr * 256:(pr + 1) * 256], cs_p[f % 2][0:32, 0:256], -1.0
                    )
                    vector.tensor_copy(
                        ssinT[32:HD, pr * 256:(pr + 1) * 256], cs_p[f % 2][32:HD, 0:256]
                    ).then_inc(sCSC, 1)
            # ones column of vaug (misc[:,0:1] memset by PL)
            vector.wait_ge(sMS, 1)
            for kt in range(NST):
                vector.tensor_copy(vaug[:, kt, HD:HD + 1], misc[:, 0:1])
            # per chunk: hsT piece copies, proj copies, rope, v copies
            for sc in range(NSC):
                sl = slice(sc * 512, (sc + 1) * 512)
                for pc in (2 * sc, 2 * sc + 1):
                    for ht in range(NHT):
                        g = pc * NHT + ht
                        vector.wait_ge(sTP, g + 1)
                        if sc >= 2 and ht == 0:
                            vector.wait_ge(sVP, sc - 1)  # hsT_ch buf free
                        vector.tensor_copy(
                            hsT_ch[sc % 2][:, ht, (pc % 2) * 256:(pc % 2 + 1) * 256],
                            tp_p[g % 2],
                        ).then_inc(sHST, 1)
                for h in range(2):
                    vector.wait_ge(sQP[h], sc + 1)
                    if sc >= 2:
                        vector.wait_ge(sSW, 16 * (2 * (3 * (sc - 2) + h) + 2))
                    vector.tensor_copy(q_ch[h][sc % 2], qp_p[h]).then_inc(sQC, 1)
                vector.wait_ge(sKP, sc + 1)
                if sc >= 2:
                    vector.wait_ge(sSW, 16 * (2 * (3 * (sc - 2) + 2) + 2))
                vector.tensor_copy(k_ch[sc % 2], kp_p).then_inc(sKC, 1)
                vector.wait_ge(sVP, sc + 1)
                vector.tensor_copy(vT[:, sl], vp_p).then_inc(sVC, 1)
                # rope: q0, q1, k
                for t in range(3):
                    vector.wait_ge(sSW, 16 * (2 * (3 * sc + t) + 2))
                    if t < 2:
                        src, ssrc, dst = q_ch[t][sc % 2], qs_ch[t][sc % 2], qT_r[t]
                    else:
                        src, ssrc, dst = k_ch[sc % 2], ks_ch[sc % 2], kT_r
                    vector.tensor_mul(tmp_a, src, cosT[:, sl])
                    vector.tensor_mul(tmp_b, ssrc, ssinT[:, sl])
                    vector.tensor_add(dst[:, sl], tmp_a, tmp_b).then_inc(sR3[t], 1)
                # v copies for this chunk
                for j in range(4):
                    kt = 4 * sc + j
                    vector.wait_ge(sVT, kt + 1)
                    vector.tensor_copy(vaug[:, kt, 0:HD], cs_p[kt % 2][0:128, 0:HD]).then_inc(sVA, 1)
            # normalize per head
            for h in range(2):
                vector.wait_ge(sPV, NST * (h + 1))
                for c in range(4):
                    n = 4 * h + c
                    sl = slice(c * 512, (c + 1) * 512)
                    if n >= 1:
                        vector.wait_ge(sRBD, 32 * (n - 1) + 16)  # misc row consumed
                    vector.reciprocal(misc[HD:HD + 1, :], pv_p[HD:HD + 1, sl]).then_inc(sRC, 1)
                    vector.wait_ge(sRBD, 32 * n + 32)
                    vector.tensor_mul(qT_r[h][:, sl], pv_p[0:HD, sl], rb_ch).then_inc(sNMC, 1)
            # out copies
            for oc in range(NSC):
                vector.wait_ge(sOP, oc + 1)
                if oc >= 2:
                    vector.wait_ge(sOD, 16 * (oc - 1))
                vector.tensor_copy(out_ch[oc % 2], op_p[oc % 2]).then_inc(sOC, 1)

        # ================= ACT: swap DMAs + exp =================
        @block.scalar
        def _(scalar):
            for sc in range(NSC):
                for t in range(3):
                    if t < 2:
                        scalar.wait_ge(sQC, 2 * sc + t + 1)
                        src, dst = q_ch[t][sc % 2], qs_ch[t][sc % 2]
                    else:
                        scalar.wait_ge(sKC, sc + 1)
                        src, dst = k_ch[sc % 2], ks_ch[sc % 2]
                    if sc >= 2:
                        scalar.wait_ge(sR3[t], sc - 1)  # qs/ks buf consumed
                    scalar.dma_start(out=dst[0:32, :], in_=src[32:HD, :]).then_inc(sSW, 16)
                    scalar.dma_start(out=dst[32:HD, :], in_=src[0:32, :]).then_inc(sSW, 16)
            # exps
            for h in range(2):
                for kt in range(NST):
                    for qh in range(2):
                        u = kt * 2 + qh
                        scalar.wait_ge(sSC, h * 32 + u + 1)
                        if h * 16 + kt >= 1:
                            scalar.wait_ge(sPV, h * 16 + kt)  # PT buf consumed
                        scalar.activation(
                            PT[u % 2][:, :],
                            sp_p[u % 2],
                            EXP, scale=0.125,
                        ).then_inc(sEX, 1)

        # ================= PL (gpsimd): memset, recip bcast, collective =================
        @block.gpsimd
        def _(gpsimd):
            gpsimd.memset(misc[:, 0:1], 1.0).then_inc(sMS, 1)
            for h in range(2):
                for c in range(4):
                    n = 4 * h + c
                    gpsimd.wait_ge(sRC, n + 1)
                    if n >= 1:
                        gpsimd.wait_ge(sNMC, n)
                    gpsimd.dma_start(
                        out=scr_d[:], in_=misc[HD:HD + 1, :]
                    ).then_inc(sRBD, 16)
                    gpsimd.wait_ge(sRBD, 32 * n + 16)
                    gpsimd.dma_start(
                        out=rb_ch,
                        in_=bass.AP(scr_d[:].tensor, 0, [[0, HD], [1, 512]]),
                    ).then_inc(sRBD, 16)
            gpsimd.wait_ge(sNMC, 8)
            gpsimd.dma_start(out=agin_d[0], in_=qT_r[0]).then_inc(sAGI, 16)
            gpsimd.dma_start(out=agin_d[1], in_=qT_r[1]).then_inc(sAGI, 16)
            gpsimd.wait_ge(sAGI, 32)
            gpsimd.collective_compute(
                "AllGather",
                mybir.AluOpType.bypass,
                replica_groups=[list(range(NCORES))],
                ins=[agin_d[:]],
                outs=[agout_d[:]],
            ).then_inc(sCC, 1)

    es.close()
    return nc


_NC_CACHE = None


def kernel(hidden_states, cos, sin, attention_mask, Wq, Wk, Wv, Wo):
    global _NC_CACHE
    if _NC_CACHE is None:
        _NC_CACHE = build_kernel()
    nc = _NC_CACHE
    hs2 = np.ascontiguousarray(np.asarray(hidden_states, dtype=np.float32).reshape(S, HID))
    cos2 = np.asarray(cos, dtype=np.float32).reshape(S, HD)
    sin2 = np.asarray(sin, dtype=np.float32).reshape(S, HD)
    cs = np.ascontiguousarray(np.concatenate([cos2, sin2], axis=1))
    Wq = np.asarray(Wq, dtype=np.float32)
    Wk = np.asarray(Wk, dtype=np.float32)
    Wv = np.asarray(Wv, dtype=np.float32)
    Wo = np.asarray(Wo, dtype=np.float32)
    ident = np.eye(128, dtype=np.float32)
    in_maps = []
    for c in range(NCORES):
        g = c // 2
        in_maps.append({
            "hs": hs2,
            "wq": np.ascontiguousarray(Wq[:, c * 128:(c + 1) * 128]),
            "wk": np.ascontiguousarray(Wk[:, g * HD:(g + 1) * HD]),
            "wv": np.ascontiguousarray(Wv[:, g * HD:(g + 1) * HD]),
            "wo": np.ascontiguousarray(Wo[:, c * 128:(c + 1) * 128]),
            "cs": cs,
            "ident": ident,
        })
    res = run_bass_kernel_spmd(nc, in_maps, core_ids=list(range(NCORES)),
                               trace=bool(int(os.environ.get("KERNEL_TRACE", "0"))))
    out = np.empty((S, HID), dtype=np.float32)
    for c in range(NCORES):
        out[:, c * 128:(c + 1) * 128] = res.results[c]["out_t"].T
    kernel.last_results = res
    return out.reshape(1, S, HID)


if __name__ == "__main__":
    import tempfile
    from concourse.bass_utils import compile_bass_kernel
    nc = build_kernel()
    with tempfile.TemporaryDirectory() as td:
        compile_bass_kernel(nc, td)
    print("COMPILE OK")



# revision 11
# speedup vs baseline: 1.8846x; 1.8846x over previous
"""GroupedQueryAttention, tensor-parallel over heads on 8 NeuronCores (raw Bass).

Core c owns q heads {2c, 2c+1} and kv head c//2. All matmul operands bf16
(f32 PSUM), inputs host-cast/transposed. Device pipeline per core:
  load hsT (host-transposed) -> qkv projections (PE, q 2-head-packed,
  k duplicated into both partition halves) -> RoPE (DVE, swap-halves via
  DMA, sign folded into host ssin table) ->
  per head h: S^T[k,q] = kT2[h*64:+64].T @ qT2[h*64:+64] (PE quadrant) ->
  P^T = exp(0.125 S^T) (ACT, bf16 out, ring of 12) ->
  pv = [V|1].T @ P^T (PE, fused denominator row) ->
  normalize: den row -> DRAM -> [64,32] recip (DVE) -> DRAM -> broadcast ->
  mul (DVE) -> AllGather per head -> o_proj accumulate both slabs (PE).
Host: out[:, c*128:(c+1)*128] = out_t_c.T.
"""
import sys, os
sys.path.insert(0, '/opt/trn_rl_repo')
import contextlib
import numpy as np
import ml_dtypes
import concourse.bass as bass
import concourse.mybir as mybir
from concourse.bass_utils import run_bass_kernel_spmd

F32 = mybir.dt.float32
BF16 = mybir.dt.bfloat16
EXP = mybir.ActivationFunctionType.Exp
NPBF = ml_dtypes.bfloat16

S, HID, HD = 2048, 1024, 64
NCORES = 8
NST = S // 128      # 16 k tiles
NHT = HID // 128    # 8 contraction tiles
NSC = 4             # 512-wide seq chunks
NPT = 16            # PT ring size (must divide 2*NST so the ring phase is
                    # uniform across the head boundary)


def build_kernel():
    nc = bass.Bass("TRN2", target_bir_lowering=False, num_devices=NCORES)

    hsT_d = nc.dram_tensor("hst", [HID, S], BF16, kind="ExternalInput")
    wq_d = nc.dram_tensor("wq", [HID, 128], BF16, kind="ExternalInput")
    wkk_d = nc.dram_tensor("wkk", [HID, 128], BF16, kind="ExternalInput")
    wv_d = nc.dram_tensor("wv", [HID, HD], BF16, kind="ExternalInput")
    wo_d = nc.dram_tensor("wo", [HID, 128], BF16, kind="ExternalInput")
    cosT_d = nc.dram_tensor("cost", [128, S], BF16, kind="ExternalInput")
    ssinT_d = nc.dram_tensor("ssint", [128, S], BF16, kind="ExternalInput")
    id_d = nc.dram_tensor("ident", [128, 128], BF16, kind="ExternalInput")
    out_d = nc.dram_tensor("out_t", [128, S], F32, kind="ExternalOutput")
    scr_d = nc.dram_tensor("scr", [2, S], F32)
    scr2_d = nc.dram_tensor("scr2", [2, S], F32)
    agin_d = nc.dram_tensor("agin", [2, HD, S], BF16)
    agout_d = nc.dram_tensor("agout", [2, NCORES, HD, S], BF16, addr_space="Shared")

    def sb(name, shape, dt):
        return nc.alloc_sbuf_tensor(name, shape, dt).ap()

    hsT = sb("hsT", [128, NHT, S], BF16)
    ident = sb("ident_sb", [128, 128], BF16)
    cosT2 = sb("cosT2", [128, S], BF16)
    ssinT2 = sb("ssinT2", [128, S], BF16)
    wq_sb = sb("wq_sb", [128, NHT, 128], BF16)
    wkk_sb = sb("wkk_sb", [128, NHT, 128], BF16)
    wv_sb = sb("wv_sb", [128, NHT, HD], BF16)
    wo_sb = sb("wo_sb", [128, NHT, 128], BF16)
    q2 = [sb(f"q2_{i}", [128, 512], BF16) for i in range(2)]
    qs2 = [sb(f"qs2_{i}", [128, 512], BF16) for i in range(2)]
    k2 = [sb(f"k2_{i}", [128, 512], BF16) for i in range(2)]
    ks2 = [sb(f"ks2_{i}", [128, 512], BF16) for i in range(2)]
    tmpa = sb("tmpa", [128, 512], BF16)
    tmpb = sb("tmpb", [128, 512], BF16)
    qT2 = sb("qT2", [128, S], BF16)
    kT2 = sb("kT2", [128, S], BF16)
    vT = sb("vT", [HD, S], BF16)
    vaug = sb("vaug", [128, NST, HD + 1], BF16)
    PT = [sb(f"PT{i}", [128, 1024], BF16) for i in range(NPT)]
    den_sb = sb("den_sb", [1, S], F32)
    den64 = sb("den64", [64, 32], F32)
    rcp64 = sb("rcp64", [64, 32], F32)
    rb = [sb(f"rb{i}", [HD, 512], F32) for i in range(2)]
    attn_sb = [sb(f"attn{h}", [HD, S], BF16) for h in range(2)]
    af = [sb(f"af{h}", [128, 4, S], BF16) for h in range(2)]
    out_ch = [sb(f"out_ch{i}", [128, 512], F32) for i in range(2)]

    ps = nc.alloc_psum_tensor("psblob", [128, 4096], F32).ap()
    qp = [ps[:, 0:512], ps[:, 512:1024]]
    kp = [ps[:, 1024:1536], ps[:, 1536:2048]]
    vp = ps[0:HD, 2048:2560]
    vtp = ps[:, 2560:3072].bitcast(BF16)          # [128, 1024] bf16: 16 x [128,64]
    sp = [ps[:, 0:1024], ps[:, 1024:2048]]
    pv = ps[0:HD + 1, 2048:4096]                  # [65, 2048]
    op = [ps[:, 0:512], ps[:, 512:1024]]

    es = contextlib.ExitStack()
    SEM = lambda n: es.enter_context(nc.semaphore(n))
    sL = SEM("sL")        # SP weight/table loads (+16)
    sHSa = SEM("sHSa")    # hsT chunks 0,2 (SP)
    sHSb = SEM("sHSb")    # hsT chunks 1,3 (gpsimd)
    sQP = SEM("sQP"); sKP = SEM("sKP"); sVP = SEM("sVP")
    sQC = SEM("sQC"); sKC = SEM("sKC"); sVC = SEM("sVC")
    sSW = SEM("sSW")      # swap DMAs (+16 each, 8/chunk)
    sQR = SEM("sQR"); sKR = SEM("sKR")
    sVT = SEM("sVT"); sVA = SEM("sVA"); sMS = SEM("sMS")
    sSC = SEM("sSC"); sEX = SEM("sEX"); sPV = SEM("sPV")
    sDNC = SEM("sDNC")    # den row copies out of psum (1/h, DVE)
    sDND = SEM("sDND")    # den chain DMAs (+16): per h: scr, den64
    sRC = SEM("sRC")      # recips (1/h)
    sDNS = SEM("sDNS")    # scr2 writes (+16/h)
    sRB = SEM("sRB")      # rb broadcasts (+16 per (h,oc))
    sNM = SEM("sNM")      # normalize muls (1 per (h,oc))
    sAG = SEM("sAG")      # agin DMAs (+16/h)
    sCC = SEM("sCC")      # collectives (1/h)
    sAF = SEM("sAF")      # af loads (+16 per (h,oc))
    sOP = SEM("sOP"); sOC = SEM("sOC"); sOD = SEM("sOD")

    def _pv(tensor, h, kt):
        tensor.wait_ge(sEX, h * 32 + 2 * kt + 2)
        if h == 1 and kt == 0:
            tensor.wait_ge(sNM, 4)  # head0 normalize done, pv psum free
        for qc in range(4):
            slot = (2 * kt + qc // 2) % NPT
            inst = tensor.matmul(
                pv[:, qc * 512:(qc + 1) * 512],
                vaug[:, kt, :],
                PT[slot][:, (qc % 2) * 512:(qc % 2 + 1) * 512],
                start=(kt == 0), stop=(kt == NST - 1),
            )
        inst.then_inc(sPV, 1)

    with nc.Block() as block:

        # ================= SP: loads + af + stores =================
        @block.sync
        def _(sync):
            sync.dma_start(out=wkk_sb, in_=wkk_d[:].rearrange("(t p) c -> p t c", p=128)).then_inc(sL, 16)
            sync.dma_start(out=wq_sb, in_=wq_d[:].rearrange("(t p) c -> p t c", p=128)).then_inc(sL, 16)
            sync.dma_start(
                out=hsT[:, :, 0:512],
                in_=hsT_d[:].rearrange("(t p) s -> p t s", p=128)[:, :, 0:512],
            ).then_inc(sHSa, 16)
            sync.dma_start(out=wv_sb, in_=wv_d[:].rearrange("(t p) c -> p t c", p=128)).then_inc(sL, 16)
            sync.dma_start(out=ident, in_=id_d[:]).then_inc(sL, 16)
            sync.dma_start(out=cosT2, in_=cosT_d[:]).then_inc(sL, 16)
            sync.dma_start(out=ssinT2, in_=ssinT_d[:]).then_inc(sL, 16)
            sync.dma_start(
                out=hsT[:, :, 1024:1536],
                in_=hsT_d[:].rearrange("(t p) s -> p t s", p=128)[:, :, 1024:1536],
            ).then_inc(sHSa, 16)
            sync.dma_start(out=wo_sb, in_=wo_d[:].rearrange("(t p) c -> p t c", p=128)).then_inc(sL, 16)
            # af loads per head slab
            for h in range(2):
                sync.wait_ge(sCC, h + 1)
                for oc in range(NSC):
                    sync.dma_start(
                        out=af[h][:, :, oc * 512:(oc + 1) * 512],
                        in_=agout_d[h].rearrange(
                            "(t a) d (n q) -> (a d) t n q", a=2, q=512
                        )[:, :, oc, :],
                    ).then_inc(sAF, 16)
            for oc in range(NSC):
                sync.wait_ge(sOC, oc + 1)
                sync.dma_start(out=out_d[:, oc * 512:(oc + 1) * 512], in_=out_ch[oc % 2]).then_inc(sOD, 16)
            sync.wait_ge(sOD, 16 * NSC)

        # ================= PE =================
        @block.tensor
        def _(tensor):
            for sc in range(NSC):
                if sc % 2 == 0:
                    tensor.wait_ge(sHSa, 16 * (sc // 2 + 1))
                else:
                    tensor.wait_ge(sHSb, 16 * (sc // 2 + 1))
                sl = slice(sc * 512, (sc + 1) * 512)
                # k projection (duplicated into both halves)
                if sc == 0:
                    tensor.wait_ge(sL, 16)
                if sc >= 2:
                    tensor.wait_ge(sKC, sc - 1)
                for ht in range(NHT):
                    inst = tensor.matmul(
                        kp[sc % 2], wkk_sb[:, ht, :], hsT[:, ht, sl],
                        start=(ht == 0), stop=(ht == NHT - 1),
                    )
                inst.then_inc(sKP, 1)
                # q projection (2 heads packed)
                if sc == 0:
                    tensor.wait_ge(sL, 32)
                if sc >= 2:
                    tensor.wait_ge(sQC, sc - 1)
                for ht in range(NHT):
                    inst = tensor.matmul(
                        qp[sc % 2], wq_sb[:, ht, :], hsT[:, ht, sl],
                        start=(ht == 0), stop=(ht == NHT - 1),
                    )
                inst.then_inc(sQP, 1)
                # v projection
                if sc == 0:
                    tensor.wait_ge(sL, 48)
                tensor.wait_ge(sVC, sc)  # vp freed by DVE copy of prior chunk
                for ht in range(NHT):
                    inst = tensor.matmul(
                        vp, wv_sb[:, ht, :], hsT[:, ht, sl],
                        start=(ht == 0), stop=(ht == NHT - 1),
                    )
                inst.then_inc(sVP, 1)
                # v transposes for this chunk's 4 ktiles
                if sc == 0:
                    tensor.wait_ge(sL, 64)
                tensor.wait_ge(sVC, sc + 1)
                for j in range(4):
                    kt = 4 * sc + j
                    inst = tensor.transpose(
                        vtp[:, kt * 64:(kt + 1) * 64],
                        vT[:, kt * 128:(kt + 1) * 128],
                        ident[0:HD, 0:HD],
                    )
                    inst.then_inc(sVT, 1)
            # ================= attention =================
            tensor.wait_ge(sQR, NSC)
            tensor.wait_ge(sKR, NSC)
            tensor.wait_ge(sVA, NST)
            tensor.wait_ge(sMS, 1)
            tensor.wait_ge(sQC, NSC)
            tensor.wait_ge(sKC, NSC)
            for h in range(2):
                hp = slice(h * 64, (h + 1) * 64)
                for kt in range(NST):
                    for qh in range(2):
                        u = h * 32 + kt * 2 + qh
                        if u >= 2:
                            tensor.wait_ge(sEX, u - 1)
                        for i in range(2):
                            inst = tensor.matmul(
                                sp[u % 2][:, i * 512:(i + 1) * 512],
                                kT2[hp, kt * 128:(kt + 1) * 128],
                                qT2[hp, qh * 1024 + i * 512: qh * 1024 + (i + 1) * 512],
                                start=True, stop=True,
                            )
                        inst.then_inc(sSC, 1)
                    if kt >= 1:
                        _pv(tensor, h, kt - 1)
                _pv(tensor, h, NST - 1)
            # ================= o_proj =================
            tensor.wait_ge(sEX, 64)
            tensor.wait_ge(sL, 112)
            for oc in range(NSC):
                tensor.wait_ge(sAF, 16 * NSC + 16 * (oc + 1))
                if oc >= 2:
                    tensor.wait_ge(sOC, oc - 1)
                for j in range(2 * NSC):
                    h, t = j // 4, j % 4
                    inst = tensor.matmul(
                        op[oc % 2], wo_sb[:, j, :],
                        af[h][:, t, oc * 512:(oc + 1) * 512],
                        start=(j == 0), stop=(j == 2 * NSC - 1),
                    )
                inst.then_inc(sOP, 1)

        # ================= DVE =================
        @block.vector
        def _(vector):
            for sc in range(NSC):
                sl = slice(sc * 512, (sc + 1) * 512)
                vector.wait_ge(sQP, sc + 1)
                if sc >= 2:
                    vector.wait_ge(sSW, 16 * (8 * (sc - 2) + 4))  # q2 swap reads done
                vector.tensor_copy(q2[sc % 2], qp[sc % 2]).then_inc(sQC, 1)
                vector.wait_ge(sKP, sc + 1)
                if sc >= 2:
                    vector.wait_ge(sSW, 16 * (8 * (sc - 2) + 8))  # k2 swap reads done
                vector.tensor_copy(k2[sc % 2], kp[sc % 2]).then_inc(sKC, 1)
                vector.wait_ge(sVP, sc + 1)
                vector.tensor_copy(vT[:, sl], vp).then_inc(sVC, 1)
                # rope q (both heads packed)
                if sc == 0:
                    vector.wait_ge(sL, 96)
                vector.wait_ge(sSW, 16 * (8 * sc + 4))
                vector.tensor_mul(tmpa, q2[sc % 2], cosT2[:, sl])
                vector.tensor_mul(tmpb, qs2[sc % 2], ssinT2[:, sl])
                vector.tensor_add(qT2[:, sl], tmpa, tmpb).then_inc(sQR, 1)
                # rope k
                vector.wait_ge(sSW, 16 * (8 * sc + 8))
                vector.tensor_mul(tmpa, k2[sc % 2], cosT2[:, sl])
                vector.tensor_mul(tmpb, ks2[sc % 2], ssinT2[:, sl])
                vector.tensor_add(kT2[:, sl], tmpa, tmpb).then_inc(sKR, 1)
                # vaug copies
                for j in range(4):
                    kt = 4 * sc + j
                    vector.wait_ge(sVT, kt + 1)
                    vector.tensor_copy(vaug[:, kt, 0:HD], vtp[:, kt * 64:(kt + 1) * 64]).then_inc(sVA, 1)
            # normalize (den row copy out of PSUM is done here: gpsimd can't
            # read PSUM)
            for h in range(2):
                vector.wait_ge(sPV, NST * (h + 1))
                if h == 1:
                    vector.wait_ge(sDND, 16)  # den_sb drained to scr (h0)
                vector.tensor_copy(den_sb, pv[HD:HD + 1, :]).then_inc(sDNC, 1)
                vector.wait_ge(sDND, 32 * h + 32)
                vector.reciprocal(rcp64, den64).then_inc(sRC, 1)
                for oc in range(NSC):
                    sl = slice(oc * 512, (oc + 1) * 512)
                    vector.wait_ge(sRB, 16 * (4 * h + oc + 1))
                    vector.tensor_mul(attn_sb[h][:, sl], pv[0:HD, sl], rb[oc % 2]).then_inc(sNM, 1)
            # out copies
            for oc in range(NSC):
                vector.wait_ge(sOP, oc + 1)
                if oc >= 2:
                    vector.wait_ge(sOD, 16 * (oc - 1))
                vector.tensor_copy(out_ch[oc % 2], op[oc % 2]).then_inc(sOC, 1)

        # ================= ACT: swap DMAs + exp =================
        @block.scalar
        def _(scalar):
            for sc in range(NSC):
                scalar.wait_ge(sQC, sc + 1)
                if sc >= 2:
                    scalar.wait_ge(sQR, sc - 1)  # qs2 buf consumed
                for b in range(2):
                    scalar.dma_start(
                        out=qs2[sc % 2][b * 64:b * 64 + 32, :],
                        in_=q2[sc % 2][b * 64 + 32:b * 64 + 64, :],
                    ).then_inc(sSW, 16)
                    scalar.dma_start(
                        out=qs2[sc % 2][b * 64 + 32:b * 64 + 64, :],
                        in_=q2[sc % 2][b * 64:b * 64 + 32, :],
                    ).then_inc(sSW, 16)
                scalar.wait_ge(sKC, sc + 1)
                if sc >= 2:
                    scalar.wait_ge(sKR, sc - 1)
                for b in range(2):
                    scalar.dma_start(
                        out=ks2[sc % 2][b * 64:b * 64 + 32, :],
                        in_=k2[sc % 2][b * 64 + 32:b * 64 + 64, :],
                    ).then_inc(sSW, 16)
                    scalar.dma_start(
                        out=ks2[sc % 2][b * 64 + 32:b * 64 + 64, :],
                        in_=k2[sc % 2][b * 64:b * 64 + 32, :],
                    ).then_inc(sSW, 16)
            # exps
            for h in range(2):
                for kt in range(NST):
                    for qh in range(2):
                        u = h * 32 + kt * 2 + qh
                        slot = (2 * kt + qh) % NPT
                        scalar.wait_ge(sSC, u + 1)
                        # PT[slot] was last written NPT//2 kt earlier (global
                        # kt order) and is consumed by that kt's PV group.
                        w = h * NST + kt - (NPT // 2 - 1)
                        if w >= 1:
                            scalar.wait_ge(sPV, w)
                        scalar.activation(
                            PT[slot][:, :], sp[u % 2], EXP, scale=0.125,
                        ).then_inc(sEX, 1)

        # ================= GPSIMD: hsT, memset, den chain, collectives ====
        @block.gpsimd
        def _(gpsimd):
            gpsimd.dma_start(
                out=hsT[:, :, 512:1024],
                in_=hsT_d[:].rearrange("(t p) s -> p t s", p=128)[:, :, 512:1024],
            ).then_inc(sHSb, 16)
            gpsimd.dma_start(
                out=hsT[:, :, 1536:2048],
                in_=hsT_d[:].rearrange("(t p) s -> p t s", p=128)[:, :, 1536:2048],
            ).then_inc(sHSb, 16)
            gpsimd.memset(vaug[:, :, HD:HD + 1], 1.0).then_inc(sMS, 1)
            for h in range(2):
                gpsimd.wait_ge(sDNC, h + 1)
                gpsimd.dma_start(out=scr_d[h], in_=den_sb).then_inc(sDND, 16)
                gpsimd.wait_ge(sDND, 32 * h + 16)
                gpsimd.dma_start(
                    out=den64, in_=scr_d[h].rearrange("(p t) -> p t", p=64)
                ).then_inc(sDND, 16)
                gpsimd.wait_ge(sRC, h + 1)
                gpsimd.dma_start(out=scr2_d[h], in_=rcp64).then_inc(sDNS, 16)
                gpsimd.wait_ge(sDNS, 16 * (h + 1))
                for oc in range(NSC):
                    if oc >= 2:
                        gpsimd.wait_ge(sNM, 4 * h + oc - 1)
                    gpsimd.dma_start(
                        out=rb[oc % 2],
                        in_=bass.AP(scr2_d[:].tensor, h * S + oc * 512, [[0, HD], [1, 512]]),
                    ).then_inc(sRB, 16)
                gpsimd.wait_ge(sNM, 4 * (h + 1))
                gpsimd.dma_start(out=agin_d[h], in_=attn_sb[h]).then_inc(sAG, 16)
                gpsimd.wait_ge(sAG, 16 * (h + 1))
                gpsimd.collective_compute(
                    "AllGather",
                    mybir.AluOpType.bypass,
                    replica_groups=[list(range(NCORES))],
                    ins=[agin_d[h]],
                    outs=[agout_d[h]],
                ).then_inc(sCC, 1)

    es.close()
    return nc


_NC_CACHE = None


def kernel(hidden_states, cos, sin, attention_mask, Wq, Wk, Wv, Wo):
    global _NC_CACHE
    if _NC_CACHE is None:
        _NC_CACHE = build_kernel()
    nc = _NC_CACHE
    hs2 = np.asarray(hidden_states, dtype=np.float32).reshape(S, HID)
    hsT = np.ascontiguousarray(hs2.T.astype(NPBF))                    # [HID, S]
    cosT = np.asarray(cos, dtype=np.float32).reshape(S, HD).T         # [64, S]
    sinT = np.asarray(sin, dtype=np.float32).reshape(S, HD).T
    ssinT = sinT.copy()
    ssinT[0:32, :] *= -1.0
    cosT2 = np.ascontiguousarray(np.concatenate([cosT, cosT], 0).astype(NPBF))
    ssinT2 = np.ascontiguousarray(np.concatenate([ssinT, ssinT], 0).astype(NPBF))
    Wq = np.asarray(Wq, dtype=np.float32)
    Wk = np.asarray(Wk, dtype=np.float32)
    Wv = np.asarray(Wv, dtype=np.float32)
    Wo = np.asarray(Wo, dtype=np.float32)
    ident = np.eye(128, dtype=np.float32).astype(NPBF)
    # slab row order for Wo: row (j, p) = (2*(2t + a) + h)*64 + d,
    # j = h*4 + t, p = a*64 + d
    order = np.empty(HID, dtype=np.int64)
    for j in range(8):
        h, t = j // 4, j % 4
        for p in range(128):
            a, d = p // 64, p % 64
            order[j * 128 + p] = (2 * (2 * t + a) + h) * 64 + d
    in_maps = []
    for c in range(NCORES):
        g = c // 2
        wk_g = Wk[:, g * HD:(g + 1) * HD]
        in_maps.append({
            "hst": hsT,
            "wq": np.ascontiguousarray(Wq[:, c * 128:(c + 1) * 128].astype(NPBF)),
            "wkk": np.ascontiguousarray(
                np.concatenate([wk_g, wk_g], axis=1).astype(NPBF)),
            "wv": np.ascontiguousarray(Wv[:, g * HD:(g + 1) * HD].astype(NPBF)),
            "wo": np.ascontiguousarray(Wo[order, c * 128:(c + 1) * 128].astype(NPBF)),
            "cost": cosT2,
            "ssint": ssinT2,
            "ident": ident,
        })
    res = run_bass_kernel_spmd(nc, in_maps, core_ids=list(range(NCORES)),
                               trace=bool(int(os.environ.get("KERNEL_TRACE", "0"))))
    out = np.empty((S, HID), dtype=np.float32)
    for c in range(NCORES):
        out[:, c * 128:(c + 1) * 128] = res.results[c]["out_t"].T
    kernel.last_results = res
    return out.reshape(1, S, HID)


if __name__ == "__main__":
    import tempfile
    from concourse.bass_utils import compile_bass_kernel
    nc = build_kernel()
    with tempfile.TemporaryDirectory() as td:
        compile_bass_kernel(nc, td)
    print("COMPILE OK")


# revision 15
# speedup vs baseline: 2.1195x; 1.1246x over previous
"""GroupedQueryAttention, tensor-parallel over heads on 8 NeuronCores (raw Bass).

Core c owns q heads {2c, 2c+1} and kv head c//2. All matmul operands bf16
(f32 PSUM), inputs host-cast/transposed/pre-arranged for contiguous DMA.
Device pipeline per core:
  load hsT (host-transposed, chunk-contiguous) -> qkv projections (PE,
  q 2-head-packed, k duplicated into both partition halves) -> RoPE (DVE;
  swap-halves DMAs: q on ACT queue, k on gpsimd queue; sign folded into
  host ssin table) ->
  per head h: S^T[k,q] = kT2[h*64:+64].T @ qT2[h*64:+64] (PE quadrant) ->
  P^T = exp(0.125 S^T) (ACT, bf16 out, ring of 16) ->
  pv = [V|1].T @ P^T (PE, fused denominator row) ->
  raw-copy pv -> SBUF (releases PSUM to next head fast) ->
  den row -> [64,32] via DMA, recip (DVE), scr2 DRAM, stride-0 broadcast,
  normalize muls (DVE) -> AllGather per head (h0's hides under h1 attn) ->
  o_proj slab0 pass during h1's collective, slab1 accumulate after.
Host: out[:, c*128:(c+1)*128] = out_t_c.T.
"""
import sys, os
sys.path.insert(0, '/opt/trn_rl_repo')
import contextlib
import numpy as np
import ml_dtypes
import concourse.bass as bass
import concourse.mybir as mybir
from concourse.bass_utils import run_bass_kernel_spmd

F32 = mybir.dt.float32
BF16 = mybir.dt.bfloat16
EXP = mybir.ActivationFunctionType.Exp
NPBF = ml_dtypes.bfloat16

S, HID, HD = 2048, 1024, 64
NCORES = 8
NST = S // 128      # 16 k tiles
NHT = HID // 128    # 8 contraction tiles
NSC = 4             # 512-wide seq chunks
NPT = 16            # PT ring size (must divide 2*NST)


def build_kernel():
    nc = bass.Bass("TRN2", target_bir_lowering=False, num_devices=NCORES)

    hsT_d = nc.dram_tensor("hst", [NSC, 128, NHT * 512], BF16, kind="ExternalInput")
    wq_d = nc.dram_tensor("wq", [128, NHT * 128], BF16, kind="ExternalInput")
    wkk_d = nc.dram_tensor("wkk", [128, NHT * 128], BF16, kind="ExternalInput")
    wv_d = nc.dram_tensor("wv", [128, NHT * HD], BF16, kind="ExternalInput")
    wo_d = nc.dram_tensor("wo", [128, NHT * 128], BF16, kind="ExternalInput")
    cosT_d = nc.dram_tensor("cost", [128, S], BF16, kind="ExternalInput")
    ssinT_d = nc.dram_tensor("ssint", [128, S], BF16, kind="ExternalInput")
    id_d = nc.dram_tensor("ident", [128, 128], BF16, kind="ExternalInput")
    out_d = nc.dram_tensor("out_t", [128, S], F32, kind="ExternalOutput")
    scr2_d = nc.dram_tensor("scr2", [2, S], F32)
    agin_d = nc.dram_tensor("agin", [2, HD, S], BF16)
    agout_d = nc.dram_tensor("agout", [2, NCORES, HD, S], BF16, addr_space="Shared")

    def sb(name, shape, dt):
        return nc.alloc_sbuf_tensor(name, shape, dt).ap()

    hsT = sb("hsT", [128, NHT, S], BF16)
    ident = sb("ident_sb", [128, 128], BF16)
    cosT2 = sb("cosT2", [128, S], BF16)
    ssinT2 = sb("ssinT2", [128, S], BF16)
    wq_sb = sb("wq_sb", [128, NHT, 128], BF16)
    wkk_sb = sb("wkk_sb", [128, NHT, 128], BF16)
    wv_sb = sb("wv_sb", [128, NHT, HD], BF16)
    wo_sb = sb("wo_sb", [128, NHT, 128], BF16)
    q2 = [sb(f"q2_{i}", [128, 512], BF16) for i in range(2)]
    qs2 = [sb(f"qs2_{i}", [128, 512], BF16) for i in range(2)]
    k2 = [sb(f"k2_{i}", [128, 512], BF16) for i in range(2)]
    ks2 = [sb(f"ks2_{i}", [128, 512], BF16) for i in range(2)]
    tmpa = sb("tmpa", [128, 512], BF16)
    tmpb = sb("tmpb", [128, 512], BF16)
    qT2 = sb("qT2", [128, S], BF16)
    kT2 = sb("kT2", [128, S], BF16)
    vT = sb("vT", [HD, S], BF16)
    vaug = sb("vaug", [128, NST, HD + 1], BF16)
    PT = [sb(f"PT{i}", [128, 1024], BF16) for i in range(NPT)]
    araw = [sb(f"araw{h}", [HD + 1, S], BF16) for h in range(2)]
    den64 = sb("den64", [64, 32], BF16)
    rcp64 = sb("rcp64", [64, 32], F32)
    rb = [sb(f"rb{i}", [HD, 512], F32) for i in range(2)]
    attn_sb = [sb(f"attn{h}", [HD, S], BF16) for h in range(2)]
    af = [sb(f"af{h}", [128, 4, S], BF16) for h in range(2)]
    out_ch = [sb(f"out_ch{i}", [128, 512], F32) for i in range(2)]

    ps = nc.alloc_psum_tensor("psblob", [128, 4096], F32).ap()
    qp = [ps[:, 0:512], ps[:, 512:1024]]
    kp = [ps[:, 1024:1536], ps[:, 1536:2048]]
    vp = [ps[0:HD, 2048:2560], ps[0:HD, 2560:3072]]
    vtp = ps[:, 3072:3584].bitcast(BF16)          # [128, 1024]: 16 x [128,64]
    sp = [ps[:, 0:1024], ps[:, 1024:2048]]
    pv = ps[0:HD + 1, 2048:4096]                  # [65, 2048]
    op4 = [ps[:, oc * 512:(oc + 1) * 512] for oc in range(NSC)]

    es = contextlib.ExitStack()
    SEM = lambda n: es.enter_context(nc.semaphore(n))
    sL = SEM("sL")        # SP weight/table loads (+16)
    sHSa = SEM("sHSa")    # hsT chunks 0,2 (SP)
    sHSb = SEM("sHSb")    # hsT chunks 1,3 (gpsimd)
    sQP = SEM("sQP"); sKP = SEM("sKP"); sVP = SEM("sVP")
    sQC = SEM("sQC"); sKC = SEM("sKC"); sVC = SEM("sVC")
    sSWQ = SEM("sSWQ")    # q swap DMAs (+16 each, 4/chunk, ACT queue)
    sSWK = SEM("sSWK")    # k swap DMAs (+16 each, 4/chunk, gpsimd queue)
    sQR = SEM("sQR"); sKR = SEM("sKR")
    sVT = SEM("sVT"); sVA = SEM("sVA"); sMS = SEM("sMS")
    sSC = SEM("sSC"); sEX = SEM("sEX"); sPV = SEM("sPV")
    sRW = SEM("sRW")      # raw attn copies out of psum (1 per (h,oc))
    sDN64 = SEM("sDN64")  # den64 DMAs (+16/h)
    sRC = SEM("sRC")      # recips (1/h)
    sDNS = SEM("sDNS")    # scr2 writes (+16/h)
    sRB = SEM("sRB")      # rb broadcasts (+16 per (h,oc))
    sNM = SEM("sNM")      # normalize muls (1 per (h,oc))
    sAG = SEM("sAG")      # agin DMAs (+16/h)
    sCC = SEM("sCC")      # collectives (1/h)
    sAF = SEM("sAF")      # af loads on SP queue: h0 x4, then h1 oc 0,1 (+16)
    sAFG = SEM("sAFG")    # af h1 oc 2,3 on gpsimd queue (+16)
    sOP = SEM("sOP")      # o_proj slab1 stop (1/oc)
    sOC = SEM("sOC"); sOD = SEM("sOD")

    def _pv(tensor, h, kt):
        tensor.wait_ge(sEX, h * 32 + 2 * kt + 2)
        if h == 1 and kt == 0:
            tensor.wait_ge(sRW, 4)  # head0 raw copies done, pv psum free
        for qc in range(4):
            slot = (2 * kt + qc // 2) % NPT
            inst = tensor.matmul(
                pv[:, qc * 512:(qc + 1) * 512],
                vaug[:, kt, :],
                PT[slot][:, (qc % 2) * 512:(qc % 2 + 1) * 512],
                start=(kt == 0), stop=(kt == NST - 1),
            )
        inst.then_inc(sPV, 1)

    with nc.Block() as block:

        # ================= SP: loads + af + stores =================
        @block.sync
        def _(sync):
            sync.dma_start(out=wkk_sb, in_=wkk_d[:]).then_inc(sL, 16)
            sync.dma_start(out=wq_sb, in_=wq_d[:]).then_inc(sL, 16)
            sync.dma_start(out=hsT[:, :, 0:512], in_=hsT_d[0]).then_inc(sHSa, 16)
            sync.dma_start(out=wv_sb, in_=wv_d[:]).then_inc(sL, 16)
            sync.dma_start(out=ident, in_=id_d[:]).then_inc(sL, 16)
            sync.dma_start(out=cosT2, in_=cosT_d[:]).then_inc(sL, 16)
            sync.dma_start(out=ssinT2, in_=ssinT_d[:]).then_inc(sL, 16)
            sync.dma_start(out=hsT[:, :, 1024:1536], in_=hsT_d[2]).then_inc(sHSa, 16)
            sync.dma_start(out=wo_sb, in_=wo_d[:]).then_inc(sL, 16)
            # af loads: h0 all 4; h1 chunks 0,1 (gpsimd does h1 chunks 2,3)
            sync.wait_ge(sCC, 1)
            for oc in range(NSC):
                sync.dma_start(
                    out=af[0][:, :, oc * 512:(oc + 1) * 512],
                    in_=agout_d[0].rearrange(
                        "(t a) d (n q) -> (a d) t n q", a=2, q=512
                    )[:, :, oc, :],
                ).then_inc(sAF, 16)
            sync.wait_ge(sCC, 2)
            for oc in range(2):
                sync.dma_start(
                    out=af[1][:, :, oc * 512:(oc + 1) * 512],
                    in_=agout_d[1].rearrange(
                        "(t a) d (n q) -> (a d) t n q", a=2, q=512
                    )[:, :, oc, :],
                ).then_inc(sAF, 16)
            for oc in range(NSC):
                sync.wait_ge(sOC, oc + 1)
                sync.dma_start(out=out_d[:, oc * 512:(oc + 1) * 512], in_=out_ch[oc % 2]).then_inc(sOD, 16)
            sync.wait_ge(sOD, 16 * NSC)

        # ================= PE =================
        @block.tensor
        def _(tensor):
            for sc in range(NSC):
                if sc % 2 == 0:
                    tensor.wait_ge(sHSa, 16 * (sc // 2 + 1))
                else:
                    tensor.wait_ge(sHSb, 16 * (sc // 2 + 1))
                sl = slice(sc * 512, (sc + 1) * 512)
                # k projection (duplicated into both halves)
                if sc == 0:
                    tensor.wait_ge(sL, 16)
                if sc >= 2:
                    tensor.wait_ge(sKC, sc - 1)
                for ht in range(NHT):
                    inst = tensor.matmul(
                        kp[sc % 2], wkk_sb[:, ht, :], hsT[:, ht, sl],
                        start=(ht == 0), stop=(ht == NHT - 1),
                    )
                inst.then_inc(sKP, 1)
                # q projection (2 heads packed)
                if sc == 0:
                    tensor.wait_ge(sL, 32)
                if sc >= 2:
                    tensor.wait_ge(sQC, sc - 1)
                for ht in range(NHT):
                    inst = tensor.matmul(
                        qp[sc % 2], wq_sb[:, ht, :], hsT[:, ht, sl],
                        start=(ht == 0), stop=(ht == NHT - 1),
                    )
                inst.then_inc(sQP, 1)
                # v projection
                if sc == 0:
                    tensor.wait_ge(sL, 48)
                if sc >= 2:
                    tensor.wait_ge(sVC, sc - 1)
                for ht in range(NHT):
                    inst = tensor.matmul(
                        vp[sc % 2], wv_sb[:, ht, :], hsT[:, ht, sl],
                        start=(ht == 0), stop=(ht == NHT - 1),
                    )
                inst.then_inc(sVP, 1)
                # v transposes for this chunk's 4 ktiles
                if sc == 0:
                    tensor.wait_ge(sL, 64)
                tensor.wait_ge(sVC, sc + 1)
                for j in range(4):
                    kt = 4 * sc + j
                    inst = tensor.transpose(
                        vtp[:, kt * 64:(kt + 1) * 64],
                        vT[:, kt * 128:(kt + 1) * 128],
                        ident[0:HD, 0:HD],
                    )
                    inst.then_inc(sVT, 1)
            # ================= attention =================
            tensor.wait_ge(sQR, NSC)
            tensor.wait_ge(sKR, NSC)
            tensor.wait_ge(sVA, NST)
            tensor.wait_ge(sMS, 1)
            tensor.wait_ge(sQC, NSC)
            tensor.wait_ge(sKC, NSC)
            tensor.wait_ge(sVC, NSC)
            for h in range(2):
                hp = slice(h * 64, (h + 1) * 64)
                for kt in range(NST):
                    for qh in range(2):
                        u = h * 32 + kt * 2 + qh
                        if u >= 2:
                            tensor.wait_ge(sEX, u - 1)
                        for i in range(2):
                            inst = tensor.matmul(
                                sp[u % 2][:, i * 512:(i + 1) * 512],
                                kT2[hp, kt * 128:(kt + 1) * 128],
                                qT2[hp, qh * 1024 + i * 512: qh * 1024 + (i + 1) * 512],
                                start=True, stop=True,
                            )
                        inst.then_inc(sSC, 1)
                    if kt >= 1:
                        _pv(tensor, h, kt - 1)
                _pv(tensor, h, NST - 1)
            # ================= o_proj (two passes over slabs) ==========
            tensor.wait_ge(sEX, 64)
            tensor.wait_ge(sL, 112)
            # slab0 pass: runs while cc1 is in flight (af0 loaded long ago)
            for oc in range(NSC):
                tensor.wait_ge(sAF, 16 * (oc + 1))
                for t in range(4):
                    tensor.matmul(
                        op4[oc], wo_sb[:, t, :],
                        af[0][:, t, oc * 512:(oc + 1) * 512],
                        start=(t == 0), stop=False, skip_group_check=True,
                    )
            # slab1 pass
            for oc in range(NSC):
                if oc < 2:
                    tensor.wait_ge(sAF, 16 * NSC + 16 * (oc + 1))   # SP queue
                else:
                    tensor.wait_ge(sAFG, 16 * (oc - 1))             # gpsimd queue
                for t in range(4):
                    inst = tensor.matmul(
                        op4[oc], wo_sb[:, 4 + t, :],
                        af[1][:, t, oc * 512:(oc + 1) * 512],
                        start=False, stop=(t == 3), skip_group_check=True,
                    )
                inst.then_inc(sOP, 1)

        # ================= DVE =================
        @block.vector
        def _(vector):
            for sc in range(NSC):
                sl = slice(sc * 512, (sc + 1) * 512)
                vector.wait_ge(sQP, sc + 1)
                if sc >= 2:
                    vector.wait_ge(sSWQ, 64 * (sc - 1))  # q2 swap reads done
                vector.tensor_copy(q2[sc % 2], qp[sc % 2]).then_inc(sQC, 1)
                vector.wait_ge(sKP, sc + 1)
                if sc >= 2:
                    vector.wait_ge(sSWK, 64 * (sc - 1))  # k2 swap reads done
                vector.tensor_copy(k2[sc % 2], kp[sc % 2]).then_inc(sKC, 1)
                vector.wait_ge(sVP, sc + 1)
                vector.tensor_copy(vT[:, sl], vp[sc % 2]).then_inc(sVC, 1)
                # rope q (both heads packed)
                if sc == 0:
                    vector.wait_ge(sL, 96)
                vector.wait_ge(sSWQ, 64 * (sc + 1))
                vector.tensor_mul(tmpa, q2[sc % 2], cosT2[:, sl])
                vector.tensor_mul(tmpb, qs2[sc % 2], ssinT2[:, sl])
                vector.tensor_add(qT2[:, sl], tmpa, tmpb).then_inc(sQR, 1)
                # rope k
                vector.wait_ge(sSWK, 64 * (sc + 1))
                vector.tensor_mul(tmpa, k2[sc % 2], cosT2[:, sl])
                vector.tensor_mul(tmpb, ks2[sc % 2], ssinT2[:, sl])
                vector.tensor_add(kT2[:, sl], tmpa, tmpb).then_inc(sKR, 1)
                # vaug copies
                for j in range(4):
                    kt = 4 * sc + j
                    vector.wait_ge(sVT, kt + 1)
                    vector.tensor_copy(vaug[:, kt, 0:HD], vtp[:, kt * 64:(kt + 1) * 64]).then_inc(sVA, 1)
            # raw copies (release pv psum) + normalize
            for h in range(2):
                vector.wait_ge(sPV, NST * (h + 1))
                for oc in range(NSC):
                    sl = slice(oc * 512, (oc + 1) * 512)
                    vector.tensor_copy(araw[h][:, sl], pv[:, sl]).then_inc(sRW, 1)
                vector.wait_ge(sDN64, 16 * (h + 1))
                if h == 1:
                    vector.wait_ge(sDNS, 16)  # rcp64 drained to scr2 (h0)
                vector.reciprocal(rcp64, den64).then_inc(sRC, 1)
                for oc in range(NSC):
                    sl = slice(oc * 512, (oc + 1) * 512)
                    vector.wait_ge(sRB, 16 * (4 * h + oc + 1))
                    vector.tensor_mul(attn_sb[h][:, sl], araw[h][0:HD, sl], rb[oc % 2]).then_inc(sNM, 1)
            # out copies
            for oc in range(NSC):
                vector.wait_ge(sOP, oc + 1)
                if oc >= 2:
                    vector.wait_ge(sOD, 16 * (oc - 1))
                vector.tensor_copy(out_ch[oc % 2], op4[oc]).then_inc(sOC, 1)

        # ================= ACT: q swap DMAs + exp =================
        @block.scalar
        def _(scalar):
            for sc in range(NSC):
                scalar.wait_ge(sQC, sc + 1)
                if sc >= 2:
                    scalar.wait_ge(sQR, sc - 1)  # qs2 buf consumed
                for b in range(2):
                    scalar.dma_start(
                        out=qs2[sc % 2][b * 64:b * 64 + 32, :],
                        in_=q2[sc % 2][b * 64 + 32:b * 64 + 64, :],
                    ).then_inc(sSWQ, 16)
                    scalar.dma_start(
                        out=qs2[sc % 2][b * 64 + 32:b * 64 + 64, :],
                        in_=q2[sc % 2][b * 64:b * 64 + 32, :],
                    ).then_inc(sSWQ, 16)
            # exps
            for h in range(2):
                for kt in range(NST):
                    for qh in range(2):
                        u = h * 32 + kt * 2 + qh
                        slot = (2 * kt + qh) % NPT
                        scalar.wait_ge(sSC, u + 1)
                        # PT[slot] was last written NPT//2 kt earlier (global
                        # kt order) and is consumed by that kt's PV group.
                        w = h * NST + kt - (NPT // 2 - 1)
                        if w >= 1:
                            scalar.wait_ge(sPV, w)
                        scalar.activation(
                            PT[slot][:, :], sp[u % 2], EXP, scale=0.125,
                        ).then_inc(sEX, 1)

        # ================= GPSIMD: hsT, memset, k swaps, den chain, cc ====
        @block.gpsimd
        def _(gpsimd):
            gpsimd.dma_start(out=hsT[:, :, 512:1024], in_=hsT_d[1]).then_inc(sHSb, 16)
            gpsimd.dma_start(out=hsT[:, :, 1536:2048], in_=hsT_d[3]).then_inc(sHSb, 16)
            gpsimd.memset(vaug[:, :, HD:HD + 1], 1.0).then_inc(sMS, 1)
            # k swap DMAs
            for sc in range(NSC):
                gpsimd.wait_ge(sKC, sc + 1)
                if sc >= 2:
                    gpsimd.wait_ge(sKR, sc - 1)
                for b in range(2):
                    gpsimd.dma_start(
                        out=ks2[sc % 2][b * 64:b * 64 + 32, :],
                        in_=k2[sc % 2][b * 64 + 32:b * 64 + 64, :],
                    ).then_inc(sSWK, 16)
                    gpsimd.dma_start(
                        out=ks2[sc % 2][b * 64 + 32:b * 64 + 64, :],
                        in_=k2[sc % 2][b * 64:b * 64 + 32, :],
                    ).then_inc(sSWK, 16)
            # den chain + collectives
            for h in range(2):
                gpsimd.wait_ge(sRW, 4 * (h + 1))
                gpsimd.dma_start(
                    out=den64,
                    in_=araw[h][HD:HD + 1, :],
                ).then_inc(sDN64, 16)
                gpsimd.wait_ge(sRC, h + 1)
                gpsimd.dma_start(out=scr2_d[h], in_=rcp64).then_inc(sDNS, 16)
                gpsimd.wait_ge(sDNS, 16 * (h + 1))
                for oc in range(NSC):
                    if oc >= 2:
                        gpsimd.wait_ge(sNM, 4 * h + oc - 1)
                    gpsimd.dma_start(
                        out=rb[oc % 2],
                        in_=bass.AP(scr2_d[:].tensor, h * S + oc * 512, [[0, HD], [1, 512]]),
                    ).then_inc(sRB, 16)
                gpsimd.wait_ge(sNM, 4 * (h + 1))
                gpsimd.dma_start(out=agin_d[h], in_=attn_sb[h]).then_inc(sAG, 16)
                gpsimd.wait_ge(sAG, 16 * (h + 1))
                gpsimd.collective_compute(
                    "AllGather",
                    mybir.AluOpType.bypass,
                    replica_groups=[list(range(NCORES))],
                    ins=[agin_d[h]],
                    outs=[agout_d[h]],
                ).then_inc(sCC, 1)
            # af h1 chunks 2,3
            gpsimd.wait_ge(sCC, 2)
            for oc in range(2, NSC):
                gpsimd.dma_start(
                    out=af[1][:, :, oc * 512:(oc + 1) * 512],
                    in_=agout_d[1].rearrange(
                        "(t a) d (n q) -> (a d) t n q", a=2, q=512
                    )[:, :, oc, :],
                ).then_inc(sAFG, 16)

    es.close()
    return nc


_NC_CACHE = None


def kernel(hidden_states, cos, sin, attention_mask, Wq, Wk, Wv, Wo):
    global _NC_CACHE
    if _NC_CACHE is None:
        _NC_CACHE = build_kernel()
    nc = _NC_CACHE
    hs2 = np.asarray(hidden_states, dtype=np.float32).reshape(S, HID)
    # hsT chunk-contiguous: [sc, p, t*512] with row (t*128+p) of hs.T
    hsT = np.ascontiguousarray(hs2.T.astype(NPBF))                    # [HID, S]
    hsT_c = np.ascontiguousarray(
        hsT.reshape(NHT, 128, NSC, 512).transpose(2, 1, 0, 3).reshape(NSC, 128, NHT * 512))
    cosT = np.asarray(cos, dtype=np.float32).reshape(S, HD).T         # [64, S]
    sinT = np.asarray(sin, dtype=np.float32).reshape(S, HD).T
    ssinT = sinT.copy()
    ssinT[0:32, :] *= -1.0
    cosT2 = np.ascontiguousarray(np.concatenate([cosT, cosT], 0).astype(NPBF))
    ssinT2 = np.ascontiguousarray(np.concatenate([ssinT, ssinT], 0).astype(NPBF))
    Wq = np.asarray(Wq, dtype=np.float32)
    Wk = np.asarray(Wk, dtype=np.float32)
    Wv = np.asarray(Wv, dtype=np.float32)
    Wo = np.asarray(Wo, dtype=np.float32)
    ident = np.eye(128, dtype=np.float32).astype(NPBF)

    def warr(w):  # [1024, X] -> [128, 8*X] partition-major contiguous
        x = w.shape[1]
        return np.ascontiguousarray(
            w.reshape(NHT, 128, x).transpose(1, 0, 2).reshape(128, NHT * x).astype(NPBF))

    # slab row order for Wo: row (j, p) = (2*(2t + a) + h)*64 + d,
    # j = h*4 + t, p = a*64 + d
    order = np.empty(HID, dtype=np.int64)
    for j in range(8):
        h, t = j // 4, j % 4
        for p in range(128):
            a, d = p // 64, p % 64
            order[j * 128 + p] = (2 * (2 * t + a) + h) * 64 + d
    in_maps = []
    for c in range(NCORES):
        g = c // 2
        wk_g = Wk[:, g * HD:(g + 1) * HD]
        in_maps.append({
            "hst": hsT_c,
            "wq": warr(Wq[:, c * 128:(c + 1) * 128]),
            "wkk": warr(np.concatenate([wk_g, wk_g], axis=1)),
            "wv": warr(Wv[:, g * HD:(g + 1) * HD]),
            "wo": np.ascontiguousarray(
                Wo[order, c * 128:(c + 1) * 128].astype(NPBF)
                .reshape(NHT, 128, 128).transpose(1, 0, 2).reshape(128, NHT * 128)),
            "cost": cosT2,
            "ssint": ssinT2,
            "ident": ident,
        })
    res = run_bass_kernel_spmd(nc, in_maps, core_ids=list(range(NCORES)),
                               trace=bool(int(os.environ.get("KERNEL_TRACE", "0"))))
    out = np.empty((S, HID), dtype=np.float32)
    for c in range(NCORES):
        out[:, c * 128:(c + 1) * 128] = res.results[c]["out_t"].T
    kernel.last_results = res
    return out.reshape(1, S, HID)


if __name__ == "__main__":
    import tempfile
    from concourse.bass_utils import compile_bass_kernel
    nc = build_kernel()
    with tempfile.TemporaryDirectory() as td:
        compile_bass_kernel(nc, td)
    print("COMPILE OK")
